# revision 13
# baseline (speedup 1.0000x reference)
"""MoE grouped-GEMM expert FFN (SwiGLU) on 8 Trainium2 NeuronCores.

Expert-parallel sharding: tokens arrive pre-grouped by expert with uniform
group size g = T/E = 1024, so core c owns experts [4c, 4c+4) and token rows
[c*4096, (c+1)*4096). No cross-core communication is needed: each core
computes its own 4 experts' FFN on its own token block.

Per-core math, per expert e:
    gu^T = w13_e^T-chunks @ x_e^T        # PE: contract H on partitions
    h^T  = silu(gate^T) * up^T           # ACT (Silu) + DVE (mul), bf16 out
    out  = h @ w2_e                      # PE: contract I on partitions

Layout/scheduling choices (each verified against the timeline cost model):
  - x is pre-transposed on the host so H lands on SBUF partitions.
  - w13 is pre-tiled into m-pair slices [mp, k, 128, 512] whose columns are
    [gate_m0 | gate_m1 | up_m0 | up_m1]; the first m-pair's k-tiles load
    fine-grained (128 KB) interleaved with the xt k-tiles so the PE starts
    ~3 us into the kernel; the remaining w13 slices and w2 load as one
    batched DMA each (HWDGE fixed cost is per dma_start).
  - Expert 0 computes m=0 and m=1 in one joint k-loop across all 8 PSUM
    banks: 1.7 us of PE work per k-tile pair vs ~1.1 us of DMA supply, so
    the PE never starves while the first weights stream in.  All other
    m-iterations run per-m with PSUM banks rotating on m parity (b0-b3 vs
    b4-b7) so epilogue i drains while iteration i+1 accumulates.
  - Phase-2 drains alternate DVE / ACT so consecutive PSUM reads overlap,
    and each output row-tile stores as one [128, 2048] DMA (the last tile
    as 4 fine DMAs to shorten the kernel-tail drain).
  - Next expert's loads issue before phase 2 so their transfers run during
    phase 2 instead of queueing behind it.
  - The SwiGLU epilogue and the final output run in bf16 (2x DVE rate,
    half the store traffic); accuracy stays ~5e-3 vs the f32 reference.
All matmuls are 128x128 stationary x [128,512] moving, bf16 in / fp32 PSUM.
"""

import sys

if "/opt/trn_rl_repo" not in sys.path:
    sys.path.insert(0, "/opt/trn_rl_repo")

import ml_dtypes
import numpy as np

import concourse.bacc as bacc
import concourse.mybir as mybir
from concourse import tile
from concourse.bass_utils import run_bass_kernel_spmd

BF16 = mybir.dt.bfloat16
F32 = mybir.dt.float32
NPBF16 = ml_dtypes.bfloat16

N_CORES = 8
E = 32
H = 2048
I = 1024
T = 32768
EPC = E // N_CORES          # experts per core = 4
G = T // E                  # tokens per expert = 1024
ROWS = EPC * G              # token rows per core = 4096
KH = H // 128               # 16 contraction tiles for GEMM1
KI = I // 128               # 8 contraction tiles for GEMM2
MP = KI // 2                # 4 m-pair output blocks in GEMM1


def build_nc():
    nc = bacc.Bacc()
    xt_d = nc.declare_dram_parameter("xt", [KH, 128, ROWS], BF16, isOutput=False)
    # w13 m-pair tiles: [e, mp, k, 128, 512] with columns
    # [gate_{2mp} | gate_{2mp+1} | up_{2mp} | up_{2mp+1}], 128 each.
    w13_d = nc.declare_dram_parameter("w13", [EPC, MP, KH, 128, 512], BF16, isOutput=False)
    w2_d = nc.declare_dram_parameter("w2", [EPC, KI, 128, H], BF16, isOutput=False)
    out_d = nc.declare_dram_parameter("out", [ROWS, H], BF16, isOutput=True)

    with tile.TileContext(nc) as tc:
        with (
            tc.tile_pool(name="xt", bufs=1) as xt_pool,
            tc.tile_pool(name="w13", bufs=1) as w13_pool,
            tc.tile_pool(name="w2", bufs=1) as w2_pool,
            tc.tile_pool(name="h", bufs=1) as h_pool,
            tc.tile_pool(name="tmp", bufs=3) as tmp_pool,
            tc.tile_pool(name="ost", bufs=2) as ost_pool,
            tc.tile_pool(name="ps", bufs=1, space="PSUM") as ps_pool,
        ):
            def bank(i, e, it):
                return ps_pool.tile([128, 512], F32, tag=f"b{i}", bufs=1,
                                    name=f"b{i}_{e}_{it}")

            def issue_loads(e):
                """Queue this expert's loads; supply order matches PE need."""
                xt_sb, w13f_sb = [], []
                for k in range(KH):
                    w = w13_pool.tile([128, 512], BF16, tag=f"w13f{k}", bufs=1,
                                      name=f"w13f{k}_{e}")
                    nc.sync.dma_start(w[:], w13_d[e, 0, k][:])
                    w13f_sb.append(w)
                    t = xt_pool.tile([128, G], BF16, tag=f"xt{k}", bufs=1,
                                     name=f"xt{k}_{e}")
                    src = xt_d[k][:, e * G:(e + 1) * G]
                    if k < 2:
                        # Half loads so the first matmuls' moving operands
                        # land ~360 ns sooner at kernel start.
                        nc.sync.dma_start(t[:, 0:512], src[:, 0:512])
                        nc.sync.dma_start(t[:, 512:1024], src[:, 512:1024])
                    else:
                        nc.sync.dma_start(t[:], src)
                    xt_sb.append(t)
                w13b_sb = []
                for mp in range(1, MP):
                    wb = w13_pool.tile([128, KH * 512], BF16, tag=f"w13b{mp}", bufs=1,
                                       name=f"w13b{mp}_{e}")
                    nc.sync.dma_start(
                        wb[:].rearrange("p (k f) -> p k f", k=KH),
                        w13_d[e, mp].rearrange("k p f -> p k f"),
                    )
                    w13b_sb.append(wb)
                w2t = w2_pool.tile([128, KI * H], BF16, tag="w2", bufs=1, name=f"w2_{e}")
                nc.sync.dma_start(
                    w2t[:].rearrange("p (k f) -> p k f", k=KI),
                    w2_d[e].rearrange("k p f -> p k f"),
                )

                def w13_slice(m, k):
                    mp, loc = divmod(m, 2)
                    if mp == 0:
                        t = w13f_sb[k]
                    else:
                        t = w13b_sb[mp - 1][:, k * 512:(k + 1) * 512]
                    return (t[:, loc * 128:(loc + 1) * 128],
                            t[:, 256 + loc * 128:256 + (loc + 1) * 128])

                def w2_slice(k, ncol):
                    return w2t[:, k * H + ncol.start:k * H + ncol.stop]

                return xt_sb, w13_slice, w2_slice

            def epilogue(e, m, pg, pu, h_sb):
                for n in range(2):
                    ncol = slice(n * 512, (n + 1) * 512)
                    tmp = tmp_pool.tile([128, 512], BF16, tag="tmp", bufs=3,
                                        name=f"tmp_{e}_{m}_{n}")
                    pu_sb = tmp_pool.tile([128, 512], BF16, tag="pusb", bufs=3,
                                          name=f"pusb_{e}_{m}_{n}")
                    nc.scalar.activation(
                        tmp[:], pg[n][:], mybir.ActivationFunctionType.Silu
                    )
                    # Both epilogue producers run on ACT so the DVE mul
                    # carries ONE merged ACT wait (the TT instruction
                    # encoding only fits a single sync-wait).
                    nc.scalar.copy(pu_sb[:], pu[n][:])
                    nc.vector.tensor_mul(h_sb[m][:, ncol], tmp[:], pu_sb[:])

            loads = issue_loads(0)
            for e in range(EPC):
                xt_sb, w13_slice, w2_slice = loads

                # Phase 1: gu^T tiles -> SwiGLU -> h^T resident in SBUF (bf16).
                h_sb = [h_pool.tile([128, G], BF16, tag=f"h{m}", bufs=1, name=f"h{m}_{e}")
                        for m in range(KI)]

                if e == 0:
                    # Joint m=0/m=1 k-loop across all 8 banks: PE consumes a
                    # k-tile pair slower than the DMA supplies the next one,
                    # so the cold start never starves the PE.
                    pgpu = [[bank((m % 2) * 4 + j, e, m) for j in range(4)]
                            for m in range(2)]
                    for k in range(KH):
                        for m in range(2):
                            wg, wu = w13_slice(m, k)
                            b = pgpu[m]
                            for n in range(2):
                                nc.tensor.matmul(
                                    b[n][:], wg, xt_sb[k][:, n * 512:(n + 1) * 512],
                                    start=(k == 0), stop=(k == KH - 1),
                                )
                            for n in range(2):
                                nc.tensor.matmul(
                                    b[2 + n][:], wu, xt_sb[k][:, n * 512:(n + 1) * 512],
                                    start=(k == 0), stop=(k == KH - 1),
                                )
                    for m in range(2):
                        epilogue(e, m, pgpu[m][:2], pgpu[m][2:], h_sb)
                    m_rest = range(2, KI)
                else:
                    m_rest = range(KI)

                for m in m_rest:
                    p = (m % 2) * 4
                    pg = [bank(p + n, e, m) for n in range(2)]
                    pu = [bank(p + 2 + n, e, m) for n in range(2)]
                    for k in range(KH):
                        wg, wu = w13_slice(m, k)
                        for n in range(2):
                            nc.tensor.matmul(
                                pg[n][:], wg, xt_sb[k][:, n * 512:(n + 1) * 512],
                                start=(k == 0), stop=(k == KH - 1),
                            )
                        for n in range(2):
                            nc.tensor.matmul(
                                pu[n][:], wu, xt_sb[k][:, n * 512:(n + 1) * 512],
                                start=(k == 0), stop=(k == KH - 1),
                            )
                    epilogue(e, m, pg, pu, h_sb)

                # Prefetch: next expert's loads go on the queue now so their
                # transfers run during phase 2 (tile reuse makes each wait
                # for its phase-1 release automatically).
                if e + 1 < EPC:
                    next_loads = issue_loads(e + 1)

                # Phase 2: out_e = h @ w2_e, streamed straight to DRAM.
                for mt in range(KI):
                    rows = slice(e * G + mt * 128, e * G + (mt + 1) * 128)
                    p = (mt % 2) * 4
                    po = [bank(p + n, e, f"o{mt}") for n in range(4)]
                    for k in range(KI):
                        hk = h_sb[k][:, mt * 128:(mt + 1) * 128]
                        for n in range(4):
                            nc.tensor.matmul(
                                po[n][:], hk, w2_slice(k, slice(n * 512, (n + 1) * 512)),
                                start=(k == 0), stop=(k == KI - 1),
                            )
                    last = e == EPC - 1 and mt == KI - 1
                    if not last:
                        ot = ost_pool.tile([128, 4 * 512], BF16, tag="ot", bufs=2,
                                           name=f"ot_{e}_{mt}")
                        for n in range(4):
                            ncol = slice(n * 512, (n + 1) * 512)
                            # Alternate drain engines so consecutive PSUM
                            # reads overlap instead of serializing on DVE.
                            if n % 2 == 0:
                                nc.vector.tensor_copy(ot[:, ncol], po[n][:])
                            else:
                                nc.scalar.copy(ot[:, ncol], po[n][:])
                        nc.sync.dma_start(out_d[rows, :], ot[:])
                    else:
                        # Kernel tail: two half tiles so each store waits
                        # only on its own drains, dispatched from separate
                        # queues so the dispatches overlap too.
                        for half in range(2):
                            oth = ost_pool.tile([128, 1024], BF16, tag=f"otl{half}",
                                                bufs=1, name=f"otl{half}")
                            for j in range(2):
                                n = half * 2 + j
                                ncol = slice(j * 512, (j + 1) * 512)
                                if n % 2 == 0:
                                    nc.vector.tensor_copy(oth[:, ncol], po[n][:])
                                else:
                                    nc.scalar.copy(oth[:, ncol], po[n][:])
                            dst = out_d[rows, half * 1024:(half + 1) * 1024]
                            if half == 0:
                                nc.sync.dma_start(dst, oth[:])
                            else:
                                nc.scalar.dma_start(dst, oth[:])

                if e + 1 < EPC:
                    loads = next_loads
    nc.compile()
    return nc


def _prep_shared(x, w2):
    # One contiguous bf16 cast each; the per-core maps then only do cheap
    # strided bf16 copies / views.
    return x.astype(NPBF16), w2.astype(NPBF16)


def _in_map_for_core(xb, w13, w2b, c):
    xs = xb[c * ROWS:(c + 1) * ROWS]                     # [4096, 2048] bf16
    xt = np.ascontiguousarray(xs.T).reshape(KH, 128, ROWS)
    w13c = w13[c * EPC:(c + 1) * EPC]                    # [4, 2048, 2048] f32
    # -> [e, mp, k, row, gu, col] so each (mp, k) tile is
    # [gate_m0 | gate_m1 | up_m0 | up_m1] over its 512 columns.
    w13t = (w13c.reshape(EPC, KH, 128, 2, MP, 256)
            .transpose(0, 4, 1, 2, 3, 5)
            .astype(NPBF16)
            .reshape(EPC, MP, KH, 128, 512))
    return {
        "xt": xt,
        "w13": w13t,
        "w2": w2b[c * EPC:(c + 1) * EPC].reshape(EPC, KI, 128, H),
    }


_NC_CACHE = []


def kernel(x, w13, w2, tokens_per_expert, decoding, _trace=False):
    x = np.asarray(x, dtype=np.float32)
    w13 = np.asarray(w13, dtype=np.float32)
    w2 = np.asarray(w2, dtype=np.float32)

    xb, w2b = _prep_shared(x, w2)
    in_maps = [_in_map_for_core(xb, w13, w2b, c) for c in range(N_CORES)]
    if not _NC_CACHE:
        _NC_CACHE.append(build_nc())
    nc = _NC_CACHE[0]
    res = run_bass_kernel_spmd(nc, in_maps, list(range(N_CORES)), trace=_trace)
    out = np.concatenate([res.results[c]["out"] for c in range(N_CORES)], axis=0)
    out = out.astype(np.float32)
    if _trace:
        return out, res
    return out
